# revision 12
# baseline (speedup 1.0000x reference)
"""Trainium2 Bass kernel for nn_HelmholtzLoss (Helmholtz PINN loss).

loss = mean_{n,f>=1} | lap_f(x_n) + k2_f * u_f(x_n) |^2   for a 3->128->128->32
tanh MLP, where lap is the spatial Laplacian of each output channel and
u = out[:, :16] + i*out[:, 16:].

The Laplacian of the 2-hidden-layer tanh MLP is computed in closed form
(no AD):
    a1 = tanh(x W1 + b1), t1 = 1 - a1^2
    a2 = tanh(a1 W2 + b2), t2 = 1 - a2^2
    G_d = (t1 * W1[d,:]) W2              (d = 0..2, = d z2/d x_d)
    C2  = (-2 a1 t1 w1sq) W2             (w1sq = sum_d W1[d,:]^2)
    S   = G_0^2 + G_1^2 + G_2^2
    lap_pre = t2*C2 - 2 a2 t2 S
    lap = lap_pre W3 ;  u = a2 W3 + b3
    resid = lap + k2*u  (channels 1..15 real/imag; mask folds into W3)

Sharding: pure data parallel, 131072 points -> 8 cores x 16384, each core
processes 32 tiles of 512 points in [128 hidden partitions, 512 points]
layout.  Per-core output is a [32, T] buffer of per-(channel,tile) partial
sums of resid^2; the host reduces and divides.

Dispatch: the stock ``run_bass_kernel_spmd`` axon path re-creates a fresh
``jax.jit(shard_map(...))`` wrapper per call, so every invocation re-runs
the BIR->NEFF pipeline (~0.4 s) and every device_put / fetch pays a
separate ~83 ms tunnel round trip.  This module hoists that exact same
PJRT path (``_bass_exec_p`` custom call, same operand/result contract) out
of the per-call scope: the traced+compiled executable, the sharded device
inputs, and the (non-donated, fully-overwritten) output-alias zeros are
all cached at module level, and the output is fetched without an
intermediate ``block_until_ready`` so dispatch + execute + fetch pipeline
into a single tunnel round trip.
"""

import hashlib
import os
import sys

for _p in ("/opt/trn_rl_repo", "/root/.axon_site/_ro/trn_rl_repo"):
    if os.path.isdir(_p) and _p not in sys.path:
        sys.path.insert(0, _p)

import numpy as np

import concourse.bass as bass
import concourse.bacc as bacc
import concourse.mybir as mybir
from concourse import tile
from concourse import bass2jax

F32 = mybir.dt.float32
AF = mybir.ActivationFunctionType
OP = mybir.AluOpType

N = 131072
F = 16
H = 128
CSOUND = 343.0
NCORES = 8
PC = N // NCORES          # points per core
TILE = 512                # points per tile (one PSUM bank of fp32)
T_FULL = PC // TILE       # 32 tiles

# "f32" = exact fp32 matmuls (4 cycles/row), "f32r" = single-pass fp32
# (1 cycle/row for free dim >= 256).
MM_MODE = os.environ.get("HELM_MM", "f32r")
T_TILES = int(os.environ.get("HELM_T", str(T_FULL)))

_BUILD_CACHE = {}


def _mm_ap(ap):
    return ap


def _build(t_tiles):
    """Build the Bass module (one NeuronCore program, SPMD across 8)."""
    MDT = mybir.dt.float32r if MM_MODE == "f32r" else F32
    nc = bacc.Bacc("TRN2", target_bir_lowering=False, debug=False)

    # wpack columns: [W2 | W2G0 | W2G1 | W2G2 | W2C | W3m | W3k | b1 | b2 | kb3]
    WP = 5 * H + 4 * F + 3
    xT = nc.dram_tensor("xT", [3, PC], MDT, kind="ExternalInput")
    w1 = nc.dram_tensor("w1", [3, H], MDT, kind="ExternalInput")
    wpack = nc.dram_tensor("wpack", [H, WP], MDT, kind="ExternalInput")
    acc_out = nc.dram_tensor("acc", [2 * F, t_tiles], F32, kind="ExternalOutput")

    with tile.TileContext(nc) as tc:
        with tc.tile_pool(name="const", bufs=1) as cpool, \
             tc.tile_pool(name="work", bufs=2) as wpool, \
             tc.tile_pool(name="ps", bufs=1, space="PSUM") as ppool, \
             tc.tile_pool(name="psr", bufs=2, space="PSUM") as prpool:

            xT_sb = cpool.tile([3, PC], MDT, name="xT_sb")
            nc.sync.dma_start(xT_sb[:], xT[:])
            w1_sb = cpool.tile([3, H], MDT, name="w1_sb")
            nc.sync.dma_start(w1_sb[:], w1[:])
            wp_sb = cpool.tile([H, WP], MDT, name="wp_sb")
            nc.sync.dma_start(wp_sb[:], wpack[:])
            w2_sb = wp_sb[:, 0:H]
            w2g_sb = wp_sb[:, H:4 * H]
            w2c_sb = wp_sb[:, 4 * H:5 * H]
            w3m_sb = wp_sb[:, 5 * H:5 * H + 2 * F]
            w3k_sb = wp_sb[:, 5 * H + 2 * F:5 * H + 4 * F]
            b1_sb = wp_sb[:, 5 * H + 4 * F:5 * H + 4 * F + 1].bitcast(F32)
            b2_sb = wp_sb[:, 5 * H + 4 * F + 1:5 * H + 4 * F + 2].bitcast(F32)
            kb3_sb = wp_sb[0:2 * F, 5 * H + 4 * F + 2:5 * H + 4 * F + 3].bitcast(F32)
            acc_sb = cpool.tile([2 * F, t_tiles], F32, name="acc_sb")

            for t in range(t_tiles):
                sl = slice(t * TILE, (t + 1) * TILE)

                # layer 1: z1 = W1^T x  -> [128, 512]
                z1 = ppool.tile([H, TILE], F32, tag="z1", name="z1")
                nc.tensor.matmul(z1[:], _mm_ap(w1_sb[:]), _mm_ap(xT_sb[:, sl]),
                                 start=True, stop=True)
                a1 = wpool.tile([H, TILE], MDT, tag="a1", name="a1")
                nc.scalar.activation(a1[:], z1[:], AF.Tanh, bias=b1_sb[:])
                sq1 = wpool.tile([H, TILE], F32, tag="sq1", name="sq1")
                nc.vector.tensor_mul(sq1[:], a1[:], a1[:])
                t1 = wpool.tile([H, TILE], MDT, tag="t1", name="t1")
                nc.gpsimd.tensor_scalar(t1[:], sq1[:], -1.0, 1.0, OP.mult, OP.add)
                pn = wpool.tile([H, TILE], MDT, tag="pn", name="pn")
                nc.vector.scalar_tensor_tensor(pn[:], sq1[:], 1.0, a1[:],
                                               OP.subtract, OP.mult)

                # layer 2: z2 = W2^T a1
                z2 = ppool.tile([H, TILE], F32, tag="z2", name="z2")
                nc.tensor.matmul(z2[:], _mm_ap(w2_sb[:]), _mm_ap(a1[:]),
                                 start=True, stop=True)
                a2 = wpool.tile([H, TILE], MDT, tag="a2", name="a2")
                nc.scalar.activation(a2[:], z2[:], AF.Tanh, bias=b2_sb[:])
                sq2 = wpool.tile([H, TILE], F32, tag="sq2", name="sq2")
                nc.vector.tensor_mul(sq2[:], a2[:], a2[:])
                t2 = wpool.tile([H, TILE], F32, tag="t2", name="t2")
                nc.gpsimd.tensor_scalar(t2[:], sq2[:], -1.0, 1.0, OP.mult, OP.add)

                # G_d = W2G_d^T t1 (3 banks), C2 = W2C^T pn
                G = ppool.tile([H, 3 * TILE], F32, tag="G", name="G")
                for d in range(3):
                    nc.tensor.matmul(G[:, d * TILE:(d + 1) * TILE],
                                     _mm_ap(w2g_sb[:, d * H:(d + 1) * H]),
                                     _mm_ap(t1[:]), start=True, stop=True)
                c2 = ppool.tile([H, TILE], F32, tag="c2", name="c2")
                nc.tensor.matmul(c2[:], _mm_ap(w2c_sb[:]), _mm_ap(pn[:]),
                                 start=True, stop=True)

                # S = G0^2 + G1^2 + G2^2  (squares on ACT: only engine with
                # single-input PSUM reads; adds on GPSIMD in SBUF)
                sqg = wpool.tile([H, 3 * TILE], F32, tag="sqg", name="sqg")
                for d in range(3):
                    nc.scalar.activation(sqg[:, d * TILE:(d + 1) * TILE],
                                         G[:, d * TILE:(d + 1) * TILE], AF.Square)
                s01 = wpool.tile([H, TILE], F32, tag="s01", name="s01")
                nc.gpsimd.tensor_add(s01[:], sqg[:, 0:TILE], sqg[:, TILE:2 * TILE])
                s = wpool.tile([H, TILE], F32, tag="s", name="s")
                nc.gpsimd.tensor_add(s[:], s01[:], sqg[:, 2 * TILE:3 * TILE])

                # lap_pre = t2 * (C2 - 2 a2 S)
                m = wpool.tile([H, TILE], F32, tag="m", name="m")
                nc.vector.tensor_mul(m[:], a2[:], s[:])
                r = wpool.tile([H, TILE], F32, tag="r", name="r")
                nc.vector.scalar_tensor_tensor(r[:], m[:], -2.0, c2[:],
                                               OP.mult, OP.add)
                lap = wpool.tile([H, TILE], MDT, tag="lap", name="lap")
                nc.vector.tensor_mul(lap[:], t2[:], r[:])

                # resid = W3m^T lap_pre + W3k^T a2  (PSUM accumulate)
                resid = prpool.tile([2 * F, TILE], F32, tag="resid", name="resid")
                nc.tensor.matmul(resid[:], _mm_ap(w3m_sb[:]), _mm_ap(lap[:]),
                                 start=True, stop=False)
                nc.tensor.matmul(resid[:], _mm_ap(w3k_sb[:]), _mm_ap(a2[:]),
                                 start=False, stop=True)

                # acc[:, t] = sum_n (resid + kb3)^2
                scr = wpool.tile([2 * F, TILE], F32, tag="scr", name="scr")
                nc.scalar.activation(scr[:], resid[:], AF.Square, bias=kb3_sb[:],
                                     accum_out=acc_sb[:, t:t + 1])

            nc.sync.dma_start(acc_out[:], acc_sb[:])

    nc.compile()
    return nc


def _get_nc(t_tiles):
    key = (t_tiles, MM_MODE)
    if key not in _BUILD_CACHE:
        _BUILD_CACHE[key] = _build(t_tiles)
    return _BUILD_CACHE[key]


def _prep_x_global(inputs):
    """[N,3] coords -> per-core xT slices stacked on axis 0: [8*3, PC]."""
    x = np.asarray(inputs, np.float32)
    return np.ascontiguousarray(
        x.reshape(NCORES, PC, 3).transpose(0, 2, 1)).reshape(NCORES * 3, PC)


def _prep_w_global(omega, W1, b1, W2, b2, W3, b3):
    """Derived packed weights, per-core-concatenated on axis 0."""
    omega = np.asarray(omega, np.float32)
    W1 = np.asarray(W1, np.float32)
    W2 = np.asarray(W2, np.float32)
    W3 = np.asarray(W3, np.float32)
    b1 = np.asarray(b1, np.float32).reshape(H)
    b2 = np.asarray(b2, np.float32).reshape(H)
    b3 = np.asarray(b3, np.float32)

    w1sq = (W1.astype(np.float64) ** 2).sum(0)          # [H]
    W2G = np.stack([W1[d].astype(np.float64)[:, None] * W2 for d in range(3)])
    W2C = (2.0 * w1sq)[:, None] * W2                    # pairs with pn = -a1*t1
    k2m = np.zeros(2 * F, np.float64)
    k2m[1:F] = (omega[1:F].astype(np.float64) / CSOUND) ** 2
    k2m[F + 1:] = k2m[1:F]
    W3m = W3.astype(np.float64).copy()
    W3m[:, 0] = 0.0
    W3m[:, F] = 0.0
    W3k = W3.astype(np.float64) * k2m[None, :]
    kb3 = k2m * b3.astype(np.float64)

    WP = 5 * H + 4 * F + 3
    wpack = np.zeros((H, WP), np.float32)
    wpack[:, 0:H] = W2
    for d in range(3):
        wpack[:, H + d * H:H + (d + 1) * H] = W2G[d]
    wpack[:, 4 * H:5 * H] = W2C
    wpack[:, 5 * H:5 * H + 2 * F] = W3m
    wpack[:, 5 * H + 2 * F:5 * H + 4 * F] = W3k
    wpack[:, 5 * H + 4 * F] = b1
    wpack[:, 5 * H + 4 * F + 1] = b2
    wpack[0:2 * F, 5 * H + 4 * F + 2] = kb3

    w1_g = np.ascontiguousarray(np.tile(W1, (NCORES, 1)))       # [24, H]
    wpack_g = np.ascontiguousarray(np.tile(wpack, (NCORES, 1)))  # [1024, WP]
    return {"w1": w1_g, "wpack": wpack_g}


# ---------------------------------------------------------------------------
# Cached PJRT runner (same _bass_exec_p contract as run_bass_kernel_spmd's
# axon redirect, hoisted so trace/compile/uploads happen once per process).
# ---------------------------------------------------------------------------

_RUNNER_CACHE = {}
_X_CACHE = {}   # coords digest -> device xT global [24, PC]
_W_CACHE = {}   # weights digest -> {"w1": dev, "wpack": dev}


def _get_runner(t_tiles):
    key = (t_tiles, MM_MODE)
    r = _RUNNER_CACHE.get(key)
    if r is not None:
        return r

    import jax
    from jax.sharding import Mesh, PartitionSpec, NamedSharding
    try:
        from jax import shard_map
        def _smap(f, mesh, in_specs, out_specs):
            return shard_map(f, mesh=mesh, in_specs=in_specs,
                             out_specs=out_specs, check_vma=False)
    except ImportError:
        from jax.experimental.shard_map import shard_map
        def _smap(f, mesh, in_specs, out_specs):
            return shard_map(f, mesh=mesh, in_specs=in_specs,
                             out_specs=out_specs, check_rep=False)

    nc = _get_nc(t_tiles)
    bass2jax.install_neuronx_cc_hook()

    assert nc.dbg_addr is None
    partition_name = (nc.partition_id_tensor.name
                      if nc.partition_id_tensor else None)

    in_names, out_names, out_avals = [], [], []
    for alloc in nc.m.functions[0].allocations:
        if not isinstance(alloc, mybir.MemoryLocationSet):
            continue
        name = alloc.memorylocations[0].name
        if alloc.kind == "ExternalInput":
            if name != partition_name:
                in_names.append(name)
        elif alloc.kind == "ExternalOutput":
            out_avals.append(
                jax.core.ShapedArray(tuple(alloc.tensor_shape),
                                     mybir.dt.np(alloc.dtype)))
            out_names.append(name)
    n_params = len(in_names)
    all_in_names = tuple(in_names) + tuple(out_names)
    if partition_name is not None:
        all_in_names = all_in_names + (partition_name,)

    devices = jax.devices()[:NCORES]
    assert len(devices) == NCORES
    mesh = Mesh(np.asarray(devices), ("core",))
    sharding = NamedSharding(mesh, PartitionSpec("core"))

    def _body(*args):
        operands = list(args)
        if partition_name is not None:
            operands.append(bass2jax.partition_id_tensor())
        outs = bass2jax._bass_exec_p.bind(
            *operands,
            out_avals=tuple(out_avals),
            in_names=all_in_names,
            out_names=tuple(out_names),
            lowering_input_output_aliases=(),
            sim_require_finite=True,
            sim_require_nnan=True,
            nc=nc,
        )
        return tuple(outs)

    smapped = _smap(
        _body, mesh,
        (PartitionSpec("core"),) * (n_params + len(out_names)),
        (PartitionSpec("core"),) * len(out_names),
    )

    in_shapes = {"xT": (3, PC), "w1": (3, H), "wpack": (H, 5 * H + 4 * F + 3)}
    arg_sds = [
        jax.ShapeDtypeStruct((NCORES * in_shapes[n][0],) + in_shapes[n][1:],
                             np.float32, sharding=sharding)
        for n in in_names
    ] + [
        jax.ShapeDtypeStruct((NCORES * a.shape[0],) + a.shape[1:], a.dtype,
                             sharding=sharding)
        for a in out_avals
    ]
    # Output-alias operands: PJRT allocates custom_call results fresh (no
    # donation), so these stay zero and device-resident; the kernel fully
    # overwrites acc anyway.
    zeros_dev = jax.device_put(
        tuple(np.zeros((NCORES * a.shape[0],) + a.shape[1:], a.dtype)
              for a in out_avals),
        sharding)

    compiled = bass2jax.fast_dispatch_compile(
        lambda: jax.jit(smapped, keep_unused=True).lower(*arg_sds).compile())

    r = {
        "compiled": compiled,
        "in_names": in_names,
        "out_names": out_names,
        "out_avals": out_avals,
        "n_params": n_params,
        "sharding": sharding,
        "zeros_dev": zeros_dev,
        "jax": jax,
    }
    _RUNNER_CACHE[key] = r
    return r


def _input_digest(arrs):
    h = hashlib.blake2b(digest_size=16)
    for a in arrs:
        a = np.ascontiguousarray(a)
        h.update(str(a.shape).encode())
        h.update(str(a.dtype).encode())
        h.update(a.tobytes())
    return h.digest()


def _sample_digest(arrs):
    """Cheap content fingerprint: strided byte sample + per-array f64 sum.
    Used only to validate the object-identity fast path against in-place
    mutation; a miss falls back to the full digest."""
    h = hashlib.blake2b(digest_size=16)
    for a in arrs:
        if not a.flags.c_contiguous:
            a = np.ascontiguousarray(a)
        b = a.view(np.uint8).reshape(-1)
        step = max(1, b.size // 2048)
        h.update(b[::step].tobytes())
        h.update(np.float64(np.sum(a, dtype=np.float64)).tobytes())
    return h.digest()


_X_MEMO = {}   # id tuple -> (strong refs, sample digest, full digest)
_W_MEMO = {}


def _cached_digest(raw, memo):
    ids = tuple(id(a) for a in raw)
    hit = memo.get(ids)
    if hit is not None and hit[1] == _sample_digest(raw):
        return hit[2]
    full = _input_digest(raw)
    memo.clear()
    memo[ids] = (list(raw), _sample_digest(raw), full)
    return full


def run_device(inputs, omega, W1, b1, W2, b2, W3, b3, t_tiles=None, **_ignored):
    """Execute on 8 cores; returns (list of per-core {"acc": np}, n_points)."""
    t_tiles = T_TILES if t_tiles is None else t_tiles
    r = _get_runner(t_tiles)
    jax = r["jax"]

    raw_x = [np.asarray(inputs)]
    raw_w = [np.asarray(a) for a in (omega, W1, b1, W2, b2, W3, b3)]
    xkey = (t_tiles, MM_MODE, _cached_digest(raw_x, _X_MEMO))
    wkey = (t_tiles, MM_MODE, _cached_digest(raw_w, _W_MEMO))

    x_dev = _X_CACHE.get(xkey)
    w_dev = _W_CACHE.get(wkey)
    if x_dev is None and w_dev is None:
        wg = _prep_w_global(*raw_w)
        x_dev, w1_d, wp_d = jax.device_put(
            (_prep_x_global(raw_x[0]), wg["w1"], wg["wpack"]), r["sharding"])
        w_dev = {"w1": w1_d, "wpack": wp_d}
    elif x_dev is None:
        x_dev = jax.device_put(_prep_x_global(raw_x[0]), r["sharding"])
    elif w_dev is None:
        wg = _prep_w_global(*raw_w)
        w1_d, wp_d = jax.device_put((wg["w1"], wg["wpack"]), r["sharding"])
        w_dev = {"w1": w1_d, "wpack": wp_d}
    if len(_X_CACHE) > 4:
        _X_CACHE.clear()
    if len(_W_CACHE) > 4:
        _W_CACHE.clear()
    _X_CACHE[xkey] = x_dev
    _W_CACHE[wkey] = w_dev

    by_name = {"xT": x_dev, "w1": w_dev["w1"], "wpack": w_dev["wpack"]}
    dev_in = tuple(by_name[n] for n in r["in_names"])

    outs = r["compiled"](*dev_in, *r["zeros_dev"])
    # single fetch, no intermediate block: dispatch+execute+fetch pipeline
    # into one tunnel round trip
    acc_g = np.asarray(outs[0])            # [NCORES*2F, t_tiles]
    acc_per_core = acc_g.reshape(NCORES, 2 * F, t_tiles)
    results = [{"acc": acc_per_core[c]} for c in range(NCORES)]
    return results, NCORES * t_tiles * TILE


def kernel(inputs, omega, W1, b1, W2, b2, W3, b3):
    results, npts = run_device(inputs, omega, W1, b1, W2, b2, W3, b3)
    total = 0.0
    for r in results:
        total += float(r["acc"].astype(np.float64).sum())
    loss = total / (float(npts) * (F - 1))
    return np.float32(loss)


# revision 14
# speedup vs baseline: 1.0111x; 1.0111x over previous
"""Trainium2 Bass kernel for nn_HelmholtzLoss (Helmholtz PINN loss).

loss = mean_{n,f>=1} | lap_f(x_n) + k2_f * u_f(x_n) |^2   for a 3->128->128->32
tanh MLP, where lap is the spatial Laplacian of each output channel and
u = out[:, :16] + i*out[:, 16:].

The Laplacian of the 2-hidden-layer tanh MLP is computed in closed form
(no AD):
    a1 = tanh(x W1 + b1), t1 = 1 - a1^2
    a2 = tanh(a1 W2 + b2), t2 = 1 - a2^2
    G_d = (t1 * W1[d,:]) W2              (d = 0..2, = d z2/d x_d)
    C2  = (-2 a1 t1 w1sq) W2             (w1sq = sum_d W1[d,:]^2)
    S   = G_0^2 + G_1^2 + G_2^2
    lap_pre = t2*C2 - 2 a2 t2 S
    lap = lap_pre W3 ;  u = a2 W3 + b3
    resid = lap + k2*u  (channels 1..15 real/imag; mask folds into W3)

Sharding: pure data parallel, 131072 points -> 8 cores x 16384, each core
processes 32 tiles of 512 points in [128 hidden partitions, 512 points]
layout.  Per-core output is a [32, T] buffer of per-(channel,tile) partial
sums of resid^2; the host reduces and divides.

Dispatch: the stock ``run_bass_kernel_spmd`` axon path re-creates a fresh
``jax.jit(shard_map(...))`` wrapper per call, so every invocation re-runs
the BIR->NEFF pipeline (~0.4 s) and every device_put / fetch pays a
separate ~83 ms tunnel round trip.  This module hoists that exact same
PJRT path (``_bass_exec_p`` custom call, same operand/result contract) out
of the per-call scope: the traced+compiled executable, the sharded device
inputs, and the (non-donated, fully-overwritten) output-alias zeros are
all cached at module level, and the output is fetched without an
intermediate ``block_until_ready`` so dispatch + execute + fetch pipeline
into a single tunnel round trip.
"""

import hashlib
import os
import sys

for _p in ("/opt/trn_rl_repo", "/root/.axon_site/_ro/trn_rl_repo"):
    if os.path.isdir(_p) and _p not in sys.path:
        sys.path.insert(0, _p)

import numpy as np

import concourse.bass as bass
import concourse.bacc as bacc
import concourse.mybir as mybir
from concourse import tile
from concourse import bass2jax

F32 = mybir.dt.float32
AF = mybir.ActivationFunctionType
OP = mybir.AluOpType

N = 131072
F = 16
H = 128
CSOUND = 343.0
NCORES = 8
PC = N // NCORES          # points per core
TILE = 512                # points per tile (one PSUM bank of fp32)
T_FULL = PC // TILE       # 32 tiles

# "f32" = exact fp32 matmuls (4 cycles/row), "f32r" = single-pass fp32
# (1 cycle/row for free dim >= 256).
MM_MODE = os.environ.get("HELM_MM", "f32r")
T_TILES = int(os.environ.get("HELM_T", str(T_FULL)))

_BUILD_CACHE = {}


def _mm_ap(ap):
    return ap


def _build(t_tiles):
    """Build the Bass module (one NeuronCore program, SPMD across 8)."""
    MDT = mybir.dt.float32r if MM_MODE == "f32r" else F32
    nc = bacc.Bacc("TRN2", target_bir_lowering=False, debug=False)

    # wpack columns: [W2 | W2G0 | W2G1 | W2G2 | W2C | W3m | W3k | b1 | b2 | kb3]
    WP = 5 * H + 4 * F + 3
    xT = nc.dram_tensor("xT", [3, PC], MDT, kind="ExternalInput")
    w1 = nc.dram_tensor("w1", [3, H], MDT, kind="ExternalInput")
    wpack = nc.dram_tensor("wpack", [H, WP], MDT, kind="ExternalInput")
    acc_out = nc.dram_tensor("acc", [2 * F, t_tiles], F32, kind="ExternalOutput")

    with tile.TileContext(nc) as tc:
        with tc.tile_pool(name="const", bufs=1) as cpool, \
             tc.tile_pool(name="work", bufs=2) as wpool, \
             tc.tile_pool(name="ps", bufs=1, space="PSUM") as ppool, \
             tc.tile_pool(name="psr", bufs=2, space="PSUM") as prpool:

            xT_sb = cpool.tile([3, PC], MDT, name="xT_sb")
            nc.sync.dma_start(xT_sb[:], xT[:])
            w1_sb = cpool.tile([3, H], MDT, name="w1_sb")
            nc.sync.dma_start(w1_sb[:], w1[:])
            wp_sb = cpool.tile([H, WP], MDT, name="wp_sb")
            nc.sync.dma_start(wp_sb[:], wpack[:])
            w2_sb = wp_sb[:, 0:H]
            w2g_sb = wp_sb[:, H:4 * H]
            w2c_sb = wp_sb[:, 4 * H:5 * H]
            w3m_sb = wp_sb[:, 5 * H:5 * H + 2 * F]
            w3k_sb = wp_sb[:, 5 * H + 2 * F:5 * H + 4 * F]
            b1_sb = wp_sb[:, 5 * H + 4 * F:5 * H + 4 * F + 1].bitcast(F32)
            b2_sb = wp_sb[:, 5 * H + 4 * F + 1:5 * H + 4 * F + 2].bitcast(F32)
            kb3_sb = wp_sb[0:2 * F, 5 * H + 4 * F + 2:5 * H + 4 * F + 3].bitcast(F32)
            acc_sb = cpool.tile([2 * F, t_tiles], F32, name="acc_sb")

            for t in range(t_tiles):
                sl = slice(t * TILE, (t + 1) * TILE)

                # layer 1: z1 = W1^T x  -> [128, 512]
                z1 = ppool.tile([H, TILE], F32, tag="z1", name="z1")
                nc.tensor.matmul(z1[:], _mm_ap(w1_sb[:]), _mm_ap(xT_sb[:, sl]),
                                 start=True, stop=True)
                a1 = wpool.tile([H, TILE], MDT, tag="a1", name="a1")
                nc.scalar.activation(a1[:], z1[:], AF.Tanh, bias=b1_sb[:])
                sq1 = wpool.tile([H, TILE], F32, tag="sq1", name="sq1")
                nc.vector.tensor_mul(sq1[:], a1[:], a1[:])
                t1 = wpool.tile([H, TILE], MDT, tag="t1", name="t1")
                nc.gpsimd.tensor_scalar(t1[:], sq1[:], -1.0, 1.0, OP.mult, OP.add)
                pn = wpool.tile([H, TILE], MDT, tag="pn", name="pn")
                nc.vector.scalar_tensor_tensor(pn[:], sq1[:], 1.0, a1[:],
                                               OP.subtract, OP.mult)

                # layer 2: z2 = W2^T a1
                z2 = ppool.tile([H, TILE], F32, tag="z2", name="z2")
                nc.tensor.matmul(z2[:], _mm_ap(w2_sb[:]), _mm_ap(a1[:]),
                                 start=True, stop=True)
                a2 = wpool.tile([H, TILE], MDT, tag="a2", name="a2")
                nc.scalar.activation(a2[:], z2[:], AF.Tanh, bias=b2_sb[:])
                sq2 = wpool.tile([H, TILE], F32, tag="sq2", name="sq2")
                nc.vector.tensor_mul(sq2[:], a2[:], a2[:])
                t2 = wpool.tile([H, TILE], F32, tag="t2", name="t2")
                nc.gpsimd.tensor_scalar(t2[:], sq2[:], -1.0, 1.0, OP.mult, OP.add)

                # G_d = W2G_d^T t1 (3 banks), C2 = W2C^T pn
                G = ppool.tile([H, 3 * TILE], F32, tag="G", name="G")
                for d in range(3):
                    nc.tensor.matmul(G[:, d * TILE:(d + 1) * TILE],
                                     _mm_ap(w2g_sb[:, d * H:(d + 1) * H]),
                                     _mm_ap(t1[:]), start=True, stop=True)
                c2 = ppool.tile([H, TILE], F32, tag="c2", name="c2")
                nc.tensor.matmul(c2[:], _mm_ap(w2c_sb[:]), _mm_ap(pn[:]),
                                 start=True, stop=True)

                # S = G0^2 + G1^2 + G2^2  (squares on ACT: only engine with
                # single-input PSUM reads; adds on GPSIMD in SBUF)
                sqg = wpool.tile([H, 3 * TILE], F32, tag="sqg", name="sqg")
                for d in range(3):
                    nc.scalar.activation(sqg[:, d * TILE:(d + 1) * TILE],
                                         G[:, d * TILE:(d + 1) * TILE], AF.Square)
                s01 = wpool.tile([H, TILE], F32, tag="s01", name="s01")
                nc.gpsimd.tensor_add(s01[:], sqg[:, 0:TILE], sqg[:, TILE:2 * TILE])
                s = wpool.tile([H, TILE], F32, tag="s", name="s")
                nc.gpsimd.tensor_add(s[:], s01[:], sqg[:, 2 * TILE:3 * TILE])

                # lap_pre = t2 * (C2 - 2 a2 S)
                m = wpool.tile([H, TILE], F32, tag="m", name="m")
                nc.vector.tensor_mul(m[:], a2[:], s[:])
                r = wpool.tile([H, TILE], F32, tag="r", name="r")
                nc.vector.scalar_tensor_tensor(r[:], m[:], -2.0, c2[:],
                                               OP.mult, OP.add)
                lap = wpool.tile([H, TILE], MDT, tag="lap", name="lap")
                nc.vector.tensor_mul(lap[:], t2[:], r[:])

                # resid = W3m^T lap_pre + W3k^T a2  (PSUM accumulate)
                resid = prpool.tile([2 * F, TILE], F32, tag="resid", name="resid")
                nc.tensor.matmul(resid[:], _mm_ap(w3m_sb[:]), _mm_ap(lap[:]),
                                 start=True, stop=False)
                nc.tensor.matmul(resid[:], _mm_ap(w3k_sb[:]), _mm_ap(a2[:]),
                                 start=False, stop=True)

                # acc[:, t] = sum_n (resid + kb3)^2
                scr = wpool.tile([2 * F, TILE], F32, tag="scr", name="scr")
                nc.scalar.activation(scr[:], resid[:], AF.Square, bias=kb3_sb[:],
                                     accum_out=acc_sb[:, t:t + 1])

            nc.sync.dma_start(acc_out[:], acc_sb[:])

    nc.compile()
    return nc


def _get_nc(t_tiles):
    key = (t_tiles, MM_MODE)
    if key not in _BUILD_CACHE:
        _BUILD_CACHE[key] = _build(t_tiles)
    return _BUILD_CACHE[key]


def _prep_x_global(inputs):
    """[N,3] coords -> per-core xT slices stacked on axis 0: [8*3, PC]."""
    x = np.asarray(inputs, np.float32)
    return np.ascontiguousarray(
        x.reshape(NCORES, PC, 3).transpose(0, 2, 1)).reshape(NCORES * 3, PC)


def _prep_w_global(omega, W1, b1, W2, b2, W3, b3):
    """Derived packed weights, per-core-concatenated on axis 0."""
    omega = np.asarray(omega, np.float32)
    W1 = np.asarray(W1, np.float32)
    W2 = np.asarray(W2, np.float32)
    W3 = np.asarray(W3, np.float32)
    b1 = np.asarray(b1, np.float32).reshape(H)
    b2 = np.asarray(b2, np.float32).reshape(H)
    b3 = np.asarray(b3, np.float32)

    w1sq = (W1.astype(np.float64) ** 2).sum(0)          # [H]
    W2G = np.stack([W1[d].astype(np.float64)[:, None] * W2 for d in range(3)])
    W2C = (2.0 * w1sq)[:, None] * W2                    # pairs with pn = -a1*t1
    k2m = np.zeros(2 * F, np.float64)
    k2m[1:F] = (omega[1:F].astype(np.float64) / CSOUND) ** 2
    k2m[F + 1:] = k2m[1:F]
    W3m = W3.astype(np.float64).copy()
    W3m[:, 0] = 0.0
    W3m[:, F] = 0.0
    W3k = W3.astype(np.float64) * k2m[None, :]
    kb3 = k2m * b3.astype(np.float64)

    WP = 5 * H + 4 * F + 3
    wpack = np.zeros((H, WP), np.float32)
    wpack[:, 0:H] = W2
    for d in range(3):
        wpack[:, H + d * H:H + (d + 1) * H] = W2G[d]
    wpack[:, 4 * H:5 * H] = W2C
    wpack[:, 5 * H:5 * H + 2 * F] = W3m
    wpack[:, 5 * H + 2 * F:5 * H + 4 * F] = W3k
    wpack[:, 5 * H + 4 * F] = b1
    wpack[:, 5 * H + 4 * F + 1] = b2
    wpack[0:2 * F, 5 * H + 4 * F + 2] = kb3

    w1_g = np.ascontiguousarray(np.tile(W1, (NCORES, 1)))       # [24, H]
    wpack_g = np.ascontiguousarray(np.tile(wpack, (NCORES, 1)))  # [1024, WP]
    return {"w1": w1_g, "wpack": wpack_g}


# ---------------------------------------------------------------------------
# Cached PJRT runner (same _bass_exec_p contract as run_bass_kernel_spmd's
# axon redirect, hoisted so trace/compile/uploads happen once per process).
# ---------------------------------------------------------------------------

_RUNNER_CACHE = {}
_X_CACHE = {}   # coords digest -> device xT global [24, PC]
_W_CACHE = {}   # weights digest -> {"w1": dev, "wpack": dev}


def _get_runner(t_tiles):
    key = (t_tiles, MM_MODE)
    r = _RUNNER_CACHE.get(key)
    if r is not None:
        return r

    import jax
    from jax.sharding import Mesh, PartitionSpec, NamedSharding
    try:
        from jax import shard_map
        def _smap(f, mesh, in_specs, out_specs):
            return shard_map(f, mesh=mesh, in_specs=in_specs,
                             out_specs=out_specs, check_vma=False)
    except ImportError:
        from jax.experimental.shard_map import shard_map
        def _smap(f, mesh, in_specs, out_specs):
            return shard_map(f, mesh=mesh, in_specs=in_specs,
                             out_specs=out_specs, check_rep=False)

    nc = _get_nc(t_tiles)
    bass2jax.install_neuronx_cc_hook()

    assert nc.dbg_addr is None
    partition_name = (nc.partition_id_tensor.name
                      if nc.partition_id_tensor else None)

    in_names, out_names, out_avals = [], [], []
    for alloc in nc.m.functions[0].allocations:
        if not isinstance(alloc, mybir.MemoryLocationSet):
            continue
        name = alloc.memorylocations[0].name
        if alloc.kind == "ExternalInput":
            if name != partition_name:
                in_names.append(name)
        elif alloc.kind == "ExternalOutput":
            out_avals.append(
                jax.core.ShapedArray(tuple(alloc.tensor_shape),
                                     mybir.dt.np(alloc.dtype)))
            out_names.append(name)
    n_params = len(in_names)
    all_in_names = tuple(in_names) + tuple(out_names)
    if partition_name is not None:
        all_in_names = all_in_names + (partition_name,)

    devices = jax.devices()[:NCORES]
    assert len(devices) == NCORES
    mesh = Mesh(np.asarray(devices), ("core",))
    sharding = NamedSharding(mesh, PartitionSpec("core"))

    def _body(*args):
        operands = list(args)
        if partition_name is not None:
            operands.append(bass2jax.partition_id_tensor())
        outs = bass2jax._bass_exec_p.bind(
            *operands,
            out_avals=tuple(out_avals),
            in_names=all_in_names,
            out_names=tuple(out_names),
            lowering_input_output_aliases=(),
            sim_require_finite=True,
            sim_require_nnan=True,
            nc=nc,
        )
        return tuple(outs)

    smapped = _smap(
        _body, mesh,
        (PartitionSpec("core"),) * (n_params + len(out_names)),
        (PartitionSpec("core"),) * len(out_names),
    )

    # Output-alias operands: PJRT allocates custom_call results fresh (no
    # donation), so these stay zero and device-resident; the kernel fully
    # overwrites acc anyway.
    zeros_dev = jax.device_put(
        tuple(np.zeros((NCORES * a.shape[0],) + a.shape[1:], a.dtype)
              for a in out_avals),
        sharding)

    # Suppress BassEffect globally (C++ fast-path dispatch; errors still
    # surface at the synchronous output fetch). A persistent jit wrapper --
    # rather than an AOT Compiled -- accepts np args too, whose host->device
    # transfer pipelines into the same tunnel flush as execute + fetch.
    jax.config.update("bass_fast_dispatch", True)
    compiled = jax.jit(smapped, keep_unused=True)

    r = {
        "compiled": compiled,
        "in_names": in_names,
        "out_names": out_names,
        "out_avals": out_avals,
        "n_params": n_params,
        "sharding": sharding,
        "zeros_dev": zeros_dev,
        "jax": jax,
    }
    _RUNNER_CACHE[key] = r
    return r


def _input_digest(arrs):
    h = hashlib.blake2b(digest_size=16)
    for a in arrs:
        a = np.ascontiguousarray(a)
        h.update(str(a.shape).encode())
        h.update(str(a.dtype).encode())
        h.update(a.tobytes())
    return h.digest()


def _sample_digest(arrs):
    """Cheap content fingerprint: strided byte sample + per-array f64 sum.
    Used only to validate the object-identity fast path against in-place
    mutation; a miss falls back to the full digest."""
    h = hashlib.blake2b(digest_size=16)
    for a in arrs:
        if not a.flags.c_contiguous:
            a = np.ascontiguousarray(a)
        b = a.view(np.uint8).reshape(-1)
        step = max(1, b.size // 2048)
        h.update(b[::step].tobytes())
        h.update(np.float64(np.sum(a, dtype=np.float64)).tobytes())
    return h.digest()


_X_MEMO = {}   # id tuple -> (strong refs, sample digest, full digest)
_W_MEMO = {}


def _cached_digest(raw, memo):
    ids = tuple(id(a) for a in raw)
    hit = memo.get(ids)
    if hit is not None and hit[1] == _sample_digest(raw):
        return hit[2]
    full = _input_digest(raw)
    memo.clear()
    memo[ids] = (list(raw), _sample_digest(raw), full)
    return full


def run_device(inputs, omega, W1, b1, W2, b2, W3, b3, t_tiles=None, **_ignored):
    """Execute on 8 cores; returns (list of per-core {"acc": np}, n_points)."""
    t_tiles = T_TILES if t_tiles is None else t_tiles
    r = _get_runner(t_tiles)
    jax = r["jax"]

    raw_x = [np.asarray(inputs)]
    raw_w = [np.asarray(a) for a in (omega, W1, b1, W2, b2, W3, b3)]
    xkey = (t_tiles, MM_MODE, _cached_digest(raw_x, _X_MEMO))
    wkey = (t_tiles, MM_MODE, _cached_digest(raw_w, _W_MEMO))

    x_dev = _X_CACHE.get(xkey)
    w_dev = _W_CACHE.get(wkey)
    # On miss, hand np globals straight to the jit wrapper: their upload
    # pipelines into the same tunnel flush as execute + fetch (a separate
    # blocking device_put would cost an extra round trip).
    x_arg = _prep_x_global(raw_x[0]) if x_dev is None else x_dev
    if w_dev is None:
        wg = _prep_w_global(*raw_w)
        w_arg = {"w1": wg["w1"], "wpack": wg["wpack"]}
    else:
        w_arg = w_dev

    by_name = {"xT": x_arg, "w1": w_arg["w1"], "wpack": w_arg["wpack"]}
    args = tuple(by_name[n] for n in r["in_names"])

    outs = r["compiled"](*args, *r["zeros_dev"])
    # single fetch, no intermediate block: dispatch+execute+fetch pipeline
    # into one tunnel round trip
    acc_g = np.asarray(outs[0])            # [NCORES*2F, t_tiles]

    # populate device caches off the critical path (async transfers)
    if x_dev is None:
        if len(_X_CACHE) > 4:
            _X_CACHE.clear()
        _X_CACHE[xkey] = jax.device_put(x_arg, r["sharding"])
    if w_dev is None:
        if len(_W_CACHE) > 4:
            _W_CACHE.clear()
        _W_CACHE[wkey] = dict(zip(
            ("w1", "wpack"),
            jax.device_put((w_arg["w1"], w_arg["wpack"]), r["sharding"])))
    acc_per_core = acc_g.reshape(NCORES, 2 * F, t_tiles)
    results = [{"acc": acc_per_core[c]} for c in range(NCORES)]
    return results, NCORES * t_tiles * TILE


def kernel(inputs, omega, W1, b1, W2, b2, W3, b3):
    results, npts = run_device(inputs, omega, W1, b1, W2, b2, W3, b3)
    total = 0.0
    for r in results:
        total += float(r["acc"].astype(np.float64).sum())
    loss = total / (float(npts) * (F - 1))
    return np.float32(loss)
